# revision 1
# baseline (speedup 1.0000x reference)
"""Trainium2 Bass kernel for causal multi-head attention.

Problem: B=2, S=2048, D=1024, H=16 heads (hd=64), fp32 in/out.
  qkv = x @ Wqkv + bqkv ; per-head causal softmax attention ; out = ctx @ Wo + bo

Sharding (8 NeuronCores): tensor-parallel over heads — 2 heads per core.
Each core computes q/k/v projections for its 2 heads (both batches), causal
attention, and its ctx^T slice [128 feat, B*S]. Four AllToAll collectives
(one per (batch, half), 256KB each) redistribute ctx^T from head-sharded to
row-sharded; each core then projects 128 rows per chunk with the full Wo.
Host reassembles the row slices.

v2 vs v1: v computed transposed with N=512 stationary-wv matmuls then
PE-transposed back (replaces 256 tiny N=128 matmuls); qkv in 4 rounds of
N=1024 matmuls; AllGather (8MB wire/core) replaced by AllToAll (0.9MB);
fused softmax normalization (reciprocal straight from PSUM, single
multiply); output projections interleaved into batch-1 attention; batch-1
windows reordered (2,3,0,1) so the cheapest chunk drains last.

Numerics: bf16 matmul operands, fp32 PSUM accumulation. Softmax uses
exp without max-subtraction (scores are ~N(0,1) after the folded 1/sqrt(hd)
scale; |s| < ~8 so fp32 exp/sums are safe). The softmax denominator comes
for free as a ones-column appended to v in the attn@v matmul.
"""

import numpy as np
import ml_dtypes

B, S, D, H, NC = 2, 2048, 1024, 16, 8
HD = D // H            # 64
HPC = H // NC          # 2 heads per core
BS = B * S             # 4096
RPB = S // NC          # 256 output rows per core per batch
KC = D // 128          # 8 contraction chunks
NR = 4                 # qkv rounds of 1024 tokens
NQT = S // 512         # 4 q-windows (512) per batch
NKT = S // 128         # 16 k-tiles (128) per batch

BF16 = ml_dtypes.bfloat16

_CACHE = {}


def _build_program():
    import concourse.bass as bass
    import concourse.mybir as mybir
    from concourse import bacc
    from concourse.tile import TileContext

    dt = mybir.dt
    f32, bf16 = dt.float32, dt.bfloat16
    ALU = mybir.AluOpType
    ACTF = mybir.ActivationFunctionType

    nc = bacc.Bacc("TRN2", target_bir_lowering=False, debug=False, num_devices=NC)

    xT = nc.dram_tensor("xT", [D, BS], bf16, kind="ExternalInput")
    wqk = nc.dram_tensor("wqk", [D, 256], bf16, kind="ExternalInput")
    wv = nc.dram_tensor("wv", [D, 128], bf16, kind="ExternalInput")
    wo = nc.dram_tensor("wo", [D, D], bf16, kind="ExternalInput")
    bqk = nc.dram_tensor("bqk", [128, 2], f32, kind="ExternalInput")
    bv = nc.dram_tensor("bv", [128, 1], f32, kind="ExternalInput")
    bo = nc.dram_tensor("bo", [128, D], f32, kind="ExternalInput")
    mask = nc.dram_tensor("mask", [128, 128], bf16, kind="ExternalInput")
    ident = nc.dram_tensor("ident", [128, 128], bf16, kind="ExternalInput")
    out = nc.dram_tensor("out", [2 * RPB, D], f32, kind="ExternalOutput")

    # collective buffers: one AllToAll per batch (fewer collectives = fewer
    # ~15-25us latencies, and only ONE lands after attention). Input is
    # shard-major: shard j = [our 128 feats, core j's 256 rows (128 per
    # half)]; the received buffer holds [core j's feats, our rows] per shard.
    ctx_dram = [nc.dram_tensor(f"ctxb{g}", [NC, 128, 2, 128], bf16)
                for g in range(B)]
    a2a_dram = [
        nc.dram_tensor(f"ctxa2a{g}", [NC, 128, 2, 128], bf16)
        for g in range(B)
    ]

    with TileContext(nc) as tc:
        with (
            tc.tile_pool(name="const", bufs=1) as cpool,
            tc.tile_pool(name="big", bufs=1) as bigpool,
            tc.tile_pool(name="xstream", bufs=2) as xpool,
            tc.tile_pool(name="vt", bufs=2) as vtpool,
            tc.tile_pool(name="exp", bufs=1) as epool,
            tc.tile_pool(name="small", bufs=2) as spool,
            tc.tile_pool(name="ag", bufs=2) as agpool,
            tc.tile_pool(name="outp", bufs=2) as opool,
            tc.tile_pool(name="psA", bufs=3, space="PSUM") as psA,   # 3x [128,1024]
            tc.tile_pool(name="psB", bufs=2, space="PSUM") as psB,   # 2x [128,512]
        ):
            # ---- constants / weights to SBUF ----
            # first x round is queued before everything else: the first
            # matmul needs only wqk + xt and gates the whole kernel
            x0 = xpool.tile([128, KC, 1024], bf16, tag="xt", name="x0")
            xT_r0 = xT.rearrange("(ko p) s -> p ko s", p=128)
            wqk_sb = cpool.tile([128, KC, 256], bf16, tag="wqk")
            nc.sync.dma_start(wqk_sb[:], wqk.rearrange("(ko p) m -> p ko m", p=128))
            nc.sync.dma_start(x0[:, :, 0:512], xT_r0[:, :, 0:512])
            nc.sync.dma_start(x0[:, :, 512:1024], xT_r0[:, :, 512:1024])
            # round 1's chunk queued ahead of the small constants so round-1
            # matmuls never wait on the DMA queue
            x1 = xpool.tile([128, KC, 1024], bf16, tag="xt", name="x1")
            nc.sync.dma_start(x1[:], xT_r0[:, :, 1024:2048])
            wv_sb = cpool.tile([128, KC, 128], bf16, tag="wv")
            nc.sync.dma_start(wv_sb[:], wv.rearrange("(ko p) m -> p ko m", p=128))
            bqk_sb = cpool.tile([128, 2], f32, tag="bqk")
            nc.sync.dma_start(bqk_sb[:], bqk[:])
            bv_sb = cpool.tile([128, 1], f32, tag="bv")
            nc.sync.dma_start(bv_sb[:], bv[:])
            mask_sb = cpool.tile([128, 128], bf16, tag="mask")
            nc.sync.dma_start(mask_sb[:], mask[:])
            ident_sb = cpool.tile([128, 128], bf16, tag="ident")
            nc.sync.dma_start(ident_sb[:], ident[:])
            wo_sb = cpool.tile([128, KC, D], bf16, tag="wo")
            bo_sb = cpool.tile([128, D], f32, tag="bo")

            # ---- persistent activations ----
            qT_sb = bigpool.tile([128, BS], bf16, tag="qT")   # [2*64 feat, B*S]
            kT_sb = bigpool.tile([128, BS], bf16, tag="kT")
            # v natural layout: [kpos, tile, head, 65] with ones at col 64
            v_sb = bigpool.tile([128, BS // 128, HPC, 65], bf16, tag="v")
            ctxT_sb = bigpool.tile([128, BS], bf16, tag="ctxT")

            nc.vector.memset(v_sb[:, :, :, 64:65], 1.0)

            xT_r = xT.rearrange("(ko p) s -> p ko s", p=128)
            vt_tiles = {}

            # ---- qkv projection: rounds of 1024 tokens ----
            def emit_qkv_round(r):
                lo = r * 1024
                if r == 0:
                    xt = x0
                elif r == 1:
                    xt = x1
                else:
                    xt = xpool.tile([128, KC, 1024], bf16, tag="xt")
                    nc.sync.dma_start(xt[:], xT_r[:, :, lo:lo + 1024])

                ps_q = psA.tile([128, 1024], f32, tag="psA", name="ps_q")
                ps_k = psA.tile([128, 1024], f32, tag="psA", name="ps_k")
                ps_v0 = psB.tile([128, 512], f32, tag="psB", name="ps_v0")
                ps_v1 = psB.tile([128, 512], f32, tag="psB", name="ps_v1")
                for kk in range(KC):
                    nc.tensor.matmul(ps_q[:, 0:512], lhsT=wqk_sb[:, kk, 0:128],
                                     rhs=xt[:, kk, 0:512],
                                     start=(kk == 0), stop=(kk == KC - 1))
                    nc.tensor.matmul(ps_q[:, 512:1024], lhsT=wqk_sb[:, kk, 0:128],
                                     rhs=xt[:, kk, 512:1024],
                                     start=(kk == 0), stop=(kk == KC - 1))
                for kk in range(KC):
                    nc.tensor.matmul(ps_k[:, 0:512], lhsT=wqk_sb[:, kk, 128:256],
                                     rhs=xt[:, kk, 0:512],
                                     start=(kk == 0), stop=(kk == KC - 1))
                    nc.tensor.matmul(ps_k[:, 512:1024], lhsT=wqk_sb[:, kk, 128:256],
                                     rhs=xt[:, kk, 512:1024],
                                     start=(kk == 0), stop=(kk == KC - 1))
                for kk in range(KC):
                    nc.tensor.matmul(ps_v0, lhsT=wv_sb[:, kk, :],
                                     rhs=xt[:, kk, 0:512],
                                     start=(kk == 0), stop=(kk == KC - 1))
                    nc.tensor.matmul(ps_v1, lhsT=wv_sb[:, kk, :],
                                     rhs=xt[:, kk, 512:1024],
                                     start=(kk == 0), stop=(kk == KC - 1))
                vt = vtpool.tile([128, 1024], bf16, tag="vT", name="vt")
                vt_tiles[r] = vt
                nc.vector.tensor_scalar_add(qT_sb[:, lo:lo + 1024], ps_q,
                                            bqk_sb[:, 0:1])
                nc.vector.tensor_scalar_add(kT_sb[:, lo:lo + 1024], ps_k,
                                            bqk_sb[:, 1:2])
                nc.vector.tensor_scalar_add(vt[:, 0:512], ps_v0,
                                            bv_sb[:, 0:1])
                nc.vector.tensor_scalar_add(vt[:, 512:1024], ps_v1,
                                            bv_sb[:, 0:1])

            # v^T [feat, tok] -> natural [tok, feat] via PE transposes,
            # 8 tiles packed per PSUM tile, drained by one strided DVE copy.
            def emit_v_transposes(r):
                vt = vt_tiles.pop(r)
                pack = psA.tile([128, 8, HPC, 64], bf16, tag="psA", name="tpack")
                for t8 in range(8):
                    c0 = t8 * 128
                    nc.tensor.transpose(pack[:, t8], vt[:, c0:c0 + 128],
                                        ident_sb[:])
                nc.vector.tensor_copy(v_sb[:, r * 8:(r + 1) * 8, :, 0:64],
                                      pack[:])

            scope1 = nc.named_scope("qkv"); scope1.__enter__()
            emit_qkv_round(0)
            emit_qkv_round(1)
            emit_v_transposes(0)
            emit_v_transposes(1)
            scope1.__exit__(None, None, None)

            scope2 = nc.named_scope("attn"); scope2.__enter__()

            def emit_a2a(b):
                # src cols of batch b viewed [half, j, s]; shard j gets its
                # two 128-row halves side by side
                nc.sync.dma_start(
                    ctx_dram[b].rearrange("j p h s -> p h j s"),
                    ctxT_sb[:, b * S:(b + 1) * S])
                nc.gpsimd.collective_compute(
                    "AllToAll",
                    mybir.AluOpType.bypass,
                    replica_groups=[list(range(NC))],
                    ins=[ctx_dram[b][:]],
                    outs=[a2a_dram[b][:]],
                )

            def emit_proj(g):
                b, half = g // 2, g % 2
                ctxag_sb = agpool.tile([128, NC, 128], bf16, tag="ctxag",
                                       name="ctxag_sb")
                nc.scalar.dma_start(
                    ctxag_sb[:],
                    a2a_dram[b].rearrange("j p h s -> p h j s")[:, half])
                ps_o0 = psB.tile([128, 512], f32, tag="psB", name="ps_o0")
                ps_o1 = psB.tile([128, 512], f32, tag="psB", name="ps_o1")
                for k in range(NC):
                    nc.tensor.matmul(ps_o0, lhsT=ctxag_sb[:, k, :],
                                     rhs=wo_sb[:, k, 0:512],
                                     start=(k == 0), stop=(k == NC - 1))
                    nc.tensor.matmul(ps_o1, lhsT=ctxag_sb[:, k, :],
                                     rhs=wo_sb[:, k, 512:1024],
                                     start=(k == 0), stop=(k == NC - 1))
                ot = opool.tile([128, D], f32, tag="ot")
                nc.vector.tensor_tensor(ot[:, 0:512], ps_o0, bo_sb[:, 0:512],
                                        ALU.add)
                nc.scalar.dma_start(out[g * 128:(g + 1) * 128, 0:512],
                                    ot[:, 0:512])
                nc.vector.tensor_tensor(ot[:, 512:1024], ps_o1,
                                        bo_sb[:, 512:1024], ALU.add)
                nc.scalar.dma_start(out[g * 128:(g + 1) * 128, 512:1024],
                                    ot[:, 512:1024])

            pending = []

            def emit_ctx(b, hl, j, exp_j):
                hp = slice(64 * hl, 64 * hl + 64)
                ps_c = psB.tile([128, 512], f32, tag="psB", name="ps_c")
                nkt = 4 * (j + 1)
                for tt in range(nkt):
                    nc.tensor.matmul(
                        ps_c[:65, :],
                        lhsT=v_sb[:, b * NKT + tt, hl, :],
                        rhs=exp_j[:, tt * 512:(tt + 1) * 512],
                        start=(tt == 0), stop=(tt == nkt - 1))
                # stage the raw ctx+den to SBUF at once: this is ps_c's only
                # consumer, so the PSUM ring frees immediately and the
                # normalize chain (recip -> gpsimd broadcast -> multiply)
                # can lag without stalling subsequent AV matmuls
                stage = spool.tile([65, 512], f32, tag="stage", bufs=3)
                nc.vector.tensor_copy(stage[:], ps_c[:65, :])
                den = spool.tile([1, 512], f32, tag="den")
                nc.vector.tensor_copy(den[:], stage[64:65, :])
                recip = spool.tile([1, 512], f32, tag="recip")
                nc.vector.reciprocal_approx_fast(out=recip[:], in_=den[:])
                bcast = spool.tile([64, 512], f32, tag="bcast", bufs=3)
                nc.gpsimd.partition_broadcast(bcast[:], recip[:])
                cs = slice(b * S + j * 512, b * S + (j + 1) * 512)
                nc.vector.tensor_tensor(ctxT_sb[hp, cs], stage[0:64, :],
                                        bcast[:], ALU.mult)
                if hl == 1 and j == 3:
                    emit_a2a(b)

            def flush_pending():
                while pending:
                    emit_ctx(*pending.pop(0))

            def emit_window(b, j):
                nkt = 4 * (j + 1)
                exp_js = []
                for hl in range(HPC):
                    exp_js.append(epool.tile([128, nkt * 512], bf16,
                                             tag=f"expj{j}h{hl}",
                                             name="exp_j"))
                if b == 0:
                    # Diagonal k-tile o covers only q >= 128*o of its window;
                    # scores/exp skip the invalid region, so zero it once.
                    # The per-(j,hl) exp buffers are reused for batch 1 and
                    # the zeros persist (ACT only ever writes valid regions).
                    for hl in range(HPC):
                        for o in range(1, 4):
                            z0 = (4 * j + o) * 512
                            nc.vector.memset(exp_js[hl][:, z0:z0 + 128 * o],
                                             0.0)
                # Scores pieces: the two heads' K=64 matmuls alternate row
                # groups (h0: rows 0:63 / h1: 64:127) so consecutive pairs
                # run concurrently in the two 64x128 PE array tiles.
                # pieces: off-diagonal k-tiles (full 512-q) in chunks of 3,
                # then the 4 diagonal tiles (ragged: tile 4j+o covers the
                # last 512-128*o q columns) packed into one PSUM tile.
                win = b * S + j * 512
                pieces = []
                tt = 0
                while tt < 4 * j:
                    npc = min(2, 4 * j - tt)
                    pieces.append([(tt + i, 0) for i in range(npc)])
                    tt += npc
                pieces.append([(4 * j, 0), (4 * j + 1, 128)])
                pieces.append([(4 * j + 2, 256), (4 * j + 3, 384)])
                for piece in pieces:
                    ps_h = [psA.tile([128, 1024], f32, tag="psA",
                                     name="ps_sc")
                            for _ in range(HPC)]
                    # pack spans so no matmul output crosses a 512-col
                    # (2KB) PSUM bank boundary
                    col = 0
                    spans = []
                    for (tile_idx, qoff) in piece:
                        w = 512 - qoff
                        if col // 512 != (col + w - 1) // 512:
                            col = ((col + 511) // 512) * 512
                        spans.append((tile_idx, qoff, col, w))
                        col += w
                    for (tile_idx, qoff, c0, w) in spans:
                        kt = b * S + tile_idx * 128
                        for hl in range(HPC):
                            hp = slice(64 * hl, 64 * hl + 64)
                            nc.tensor.matmul(
                                ps_h[hl][:, c0:c0 + w],
                                lhsT=kT_sb[hp, kt:kt + 128],
                                rhs=qT_sb[hp, win + qoff:win + 512],
                                start=True, stop=True)
                    # exp: one ACT op per head per src/dst-contiguous run
                    for hl in range(HPC):
                        run = []
                        for (tile_idx, qoff, c0, w) in spans:
                            dst = tile_idx * 512 + qoff
                            if run and run[-1][1] + run[-1][2] == dst \
                                    and run[-1][0] + run[-1][2] == c0:
                                run[-1] = (run[-1][0], run[-1][1],
                                           run[-1][2] + w)
                            else:
                                run.append((c0, dst, w))
                        for (c0, dst, w) in run:
                            nc.scalar.activation(
                                exp_js[hl][:, dst:dst + w],
                                ps_h[hl][:, c0:c0 + w], ACTF.Exp)
                # triangular causal mask on each diagonal tile's first
                # 128 valid columns
                for hl in range(HPC):
                    for o in range(4):
                        lo = (4 * j + o) * 512 + 128 * o
                        nc.vector.tensor_tensor(exp_js[hl][:, lo:lo + 128],
                                                exp_js[hl][:, lo:lo + 128],
                                                mask_sb[:],
                                                ALU.mult)
                # lag-1 ctx emission (both heads of the previous window)
                for hl in range(HPC):
                    pending.append((b, hl, j, exp_js[hl]))
                while len(pending) > 2:
                    emit_ctx(*pending.pop(0))

            # batch 0: natural window order; qkv rounds 2/3 + v transposes
            # interleave to keep the PE fed while ACT drains exp.
            # all x DMAs must precede the first collective trigger on the
            # Sync queue, so round 3 is emitted before window (0,2)
            emit_window(0, 0)
            emit_qkv_round(2)
            # wo/bo are needed only by the projections (~180us in); queued
            # here so rounds 2/3's x chunks never sit behind them
            nc.sync.dma_start(wo_sb[:], wo.rearrange("(ko p) m -> p ko m", p=128))
            nc.sync.dma_start(bo_sb[:], bo[:])
            emit_window(0, 1)
            emit_v_transposes(2)
            emit_qkv_round(3)
            emit_window(0, 2)
            emit_v_transposes(3)
            emit_window(0, 3)
            # batch 1: natural window order staggers the AllToAll issues on
            # the (in-order) CC queue — a2a(2) launches mid-batch and hides
            # under windows 2/3; only a2a(3) lands at the end. proj(0) fills
            # the ACT-bound stretch of window 3; the remaining projections
            # cover a2a(3)'s latency, with proj(3) (its consumer) last.
            emit_window(1, 0)
            emit_window(1, 1)
            emit_window(1, 2)
            emit_window(1, 3)
            flush_pending()
            emit_proj(0)
            emit_proj(1)
            emit_proj(2)
            emit_proj(3)

            scope2.__exit__(None, None, None)

    nc.compile()
    return nc


def _prep_inputs(x, Wqkv, bqkv, Wo, bo):
    x = np.asarray(x, dtype=np.float32)
    Wqkv = np.asarray(Wqkv, dtype=np.float32)
    bqkv = np.asarray(bqkv, dtype=np.float32)
    Wo = np.asarray(Wo, dtype=np.float32)
    bo = np.asarray(bo, dtype=np.float32)

    xT = np.ascontiguousarray(x.reshape(BS, D).T).astype(BF16)
    wo_b = Wo.astype(BF16)
    bo_t = np.tile(bo.astype(np.float32), (128, 1))

    kp = np.arange(128)[:, None]
    u = np.arange(128)[None, :]
    mask = (u >= kp).astype(BF16)
    ident = np.eye(128, dtype=BF16)

    scale = np.float32(1.0 / np.sqrt(HD))

    # Wqkv columns per head h: q = 192h..+64, k = +64, v = +128
    W3 = Wqkv.reshape(D, H, 3, HD)
    b3 = bqkv.reshape(H, 3, HD)

    in_maps = []
    for c in range(NC):
        hs = [HPC * c + i for i in range(HPC)]
        wq = np.concatenate([W3[:, h, 0, :] for h in hs], axis=1) * scale
        wk = np.concatenate([W3[:, h, 1, :] for h in hs], axis=1)
        wv_ = np.concatenate([W3[:, h, 2, :] for h in hs], axis=1)
        bq = np.concatenate([b3[h, 0, :] for h in hs]) * scale
        bk = np.concatenate([b3[h, 1, :] for h in hs])
        bv_ = np.concatenate([b3[h, 2, :] for h in hs])
        in_maps.append({
            "xT": xT,
            "wqk": np.ascontiguousarray(
                np.concatenate([wq, wk], axis=1)).astype(BF16),
            "wv": np.ascontiguousarray(wv_).astype(BF16),
            "wo": wo_b,
            "bqk": np.ascontiguousarray(
                np.stack([bq, bk], axis=1)).astype(np.float32),
            "bv": bv_.astype(np.float32).reshape(128, 1),
            "bo": bo_t,
            "mask": mask,
            "ident": ident,
        })
    return in_maps


def run(x, Wqkv, bqkv, Wo, bo, trace=False):
    from concourse.bass_utils import run_bass_kernel_spmd

    if "nc" not in _CACHE:
        _CACHE["nc"] = _build_program()
    nc = _CACHE["nc"]
    in_maps = _prep_inputs(x, Wqkv, bqkv, Wo, bo)
    res = run_bass_kernel_spmd(nc, in_maps, list(range(NC)), trace=trace)
    # core c returns [512, D]: 4 chunks of 128 rows: (b0 rows 128c..),
    # (b0 rows 1024+128c..), (b1 rows 128c..), (b1 rows 1024+128c..)
    full = np.empty((B, S, D), dtype=np.float32)
    for c in range(NC):
        r = res.results[c]["out"]
        for g in range(4):
            b, half = g // 2, g % 2
            lo = half * 1024 + 128 * c
            full[b, lo:lo + 128, :] = r[g * 128:(g + 1) * 128, :]
    return full, res


def kernel(x, Wqkv, bqkv, Wo, bo):
    out, _ = run(x, Wqkv, bqkv, Wo, bo)
    return out

